# revision 5
# baseline (speedup 1.0000x reference)
"""Trainium2 Bass kernel for nn_AdaptiveNoisingModule (retrieval_knn).

Strategy (8 NeuronCores, data-parallel over B*N rows):
  - each core owns 1024 of the 8192 query rows; memory bank replicated
  - scores s = 2*x.m - |m|^2 computed as one augmented fp32 matmul
    (top-9 smallest distances == top-9 largest s); bank streamed once,
    chunk-of-512 at a time, all 8 row-tiles share each bank chunk (PSUM-resident)
  - streaming top-k: per chunk, VectorE max/max_index reduce 512 cols -> 8
    candidates; 256 candidates per row merged at the end via
    max/match_replace/max + scalar_tensor_tensor(is_equal)*idx extraction
  - dma_gather fetches the 9 neighbor bank vectors per row for the exact
    gradient; influence/noise_std computed on-chip
  - one tiny AllReduce (8 cores) for the global distance-signal mean/std
"""

import numpy as np

B, N, D, M = 8, 1024, 1024, 16384
KNN = 9
NCORES = 8
ROWS = B * N                  # 8192
RPC = ROWS // NCORES          # 1024 rows per core
RT = RPC // 128               # 8 row-tiles of 128
CH = 512                      # bank chunk (1 PSUM bank of fp32)
NCH = M // CH                 # 32 chunks
KD = D + 1                    # augmented contraction dim (adds -|m|^2 row)
NKS = (KD + 127) // 128       # 9 k-steps (last has 1 partition)
NCAND = NCH * 8               # 256 candidates per row
GG = 3                        # gather group: neighbors per dma_gather
MIN_STD, MAX_STD, EPS = 0.01, 0.5, 1e-8

_cache = {}


def _build_nc():
    import concourse.bass as bass
    import concourse.bacc as bacc
    import concourse.mybir as mybir
    import concourse.tile as tile

    f32 = mybir.dt.float32
    i16 = mybir.dt.int16
    u16 = mybir.dt.uint16
    Alu = mybir.AluOpType
    Act = mybir.ActivationFunctionType
    Ax = mybir.AxisListType

    nc = bacc.Bacc("TRN2", target_bir_lowering=False, debug=False,
                   num_devices=NCORES)

    # ---- I/O ----
    xa_d = nc.dram_tensor("xa", [KD, RPC], f32, kind="ExternalInput")
    xn_d = nc.dram_tensor("xnat", [RPC, D], f32, kind="ExternalInput")
    mta_d = nc.dram_tensor("mta", [KD, M], f32, kind="ExternalInput")
    bank_d = nc.dram_tensor("bank", [M, D], f32, kind="ExternalInput")
    iw_d = nc.dram_tensor("iw", [128, D], f32, kind="ExternalInput")
    dw_d = nc.dram_tensor("dw", [128, 1], f32, kind="ExternalInput")
    cb_d = nc.dram_tensor("cbase", [128, NCAND], f32, kind="ExternalInput")

    inf_d = nc.dram_tensor("influence", [RPC, D], f32, kind="ExternalOutput")
    noi_d = nc.dram_tensor("noise", [RPC, D], f32, kind="ExternalOutput")
    dst_d = nc.dram_tensor("dist", [RPC, KNN], f32, kind="ExternalOutput")

    with tile.TileContext(nc) as tc:
        with (
            tc.tile_pool(name="const", bufs=1) as cp,
            tc.tile_pool(name="mt", bufs=2) as mtp,
            tc.tile_pool(name="sch", bufs=4) as schp,
            tc.tile_pool(name="cand", bufs=1) as candp,
            tc.tile_pool(name="work", bufs=1) as wp,
            tc.tile_pool(name="gm", bufs=2) as gmp,
            tc.tile_pool(name="ps", bufs=1, space="PSUM") as psp,
            tc.tile_pool(name="dram", bufs=1, space="DRAM") as dramp,
        ):
            # ---- constants / resident tensors ----
            xa_sb = []
            for k in range(NKS):
                p = min(128, KD - k * 128)
                t = cp.tile([p, RPC], f32, tag=f"xa{k}", name=f"xa{k}")
                nc.sync.dma_start(t[:], xa_d[k * 128:k * 128 + p, :])
                xa_sb.append(t)
            iw_sb = cp.tile([128, D], f32, tag="iw")
            nc.sync.dma_start(iw_sb[:], iw_d[:])
            dw_sb = cp.tile([128, 1], f32, tag="dw")
            nc.sync.dma_start(dw_sb[:], dw_d[:])
            cb_sb = cp.tile([128, NCAND], f32, tag="cb")
            nc.sync.dma_start(cb_sb[:], cb_d[:])
            ones_sb = cp.tile([128, 1], f32, tag="ones")
            nc.vector.memset(ones_sb[:], 1.0)

            # per-row-tile stores that live across phases
            cand_v = [candp.tile([128, NCAND], f32, tag=f"cv{rt}", name=f"cv{rt}")
                      for rt in range(RT)]
            cand_li = [candp.tile([128, NCAND], u16, tag=f"cl{rt}", name=f"cl{rt}")
                       for rt in range(RT)]
            inf_sb = [candp.tile([128, D], f32, tag=f"inf{rt}", name=f"inf{rt}")
                      for rt in range(RT)]
            d9_all = candp.tile([128, RT * KNN], f32, tag="d9all")
            im_all = candp.tile([128, RT], f32, tag="imall")
            ii_all = candp.tile([128, RT], f32, tag="iiall")
            dstat = candp.tile([128, 2 * RT], f32, tag="dstat")

            # ---- phase 1: streaming matmul + per-chunk top-8 ----
            for n in range(NCH):
                mts = []
                for k in range(NKS):
                    p = min(128, KD - k * 128)
                    mt = mtp.tile([p, CH], f32, tag=f"mt{k}", name=f"mt{k}")
                    nc.sync.dma_start(mt[:], mta_d[k * 128:k * 128 + p,
                                                   n * CH:(n + 1) * CH])
                    mts.append(mt)
                for rt in range(RT):
                    ps = psp.tile([128, CH], f32, tag=f"ps{rt}", name=f"ps{rt}")
                    for k in range(NKS):
                        nc.tensor.matmul(
                            ps[:], xa_sb[k][:, rt * 128:(rt + 1) * 128], mts[k][:],
                            start=(k == 0), stop=(k == NKS - 1))
                    sch = schp.tile([128, CH], f32, tag="sch")
                    nc.scalar.copy(sch[:], ps[:])
                    nc.vector.max(cand_v[rt][:, n * 8:(n + 1) * 8], sch[:])
                    nc.vector.max_index(cand_li[rt][:, n * 8:(n + 1) * 8],
                                        cand_v[rt][:, n * 8:(n + 1) * 8], sch[:])

            # ---- phase 2: per row-tile merge, gather, gradient ----
            for rt in range(RT):
                d9 = d9_all[:, rt * KNN:(rt + 1) * KNN]

                # x natural + x2
                xn_rt = wp.tile([128, D], f32, tag="xnrt")
                nc.sync.dma_start(xn_rt[:], xn_d[rt * 128:(rt + 1) * 128, :])
                xsq = wp.tile([128, D], f32, tag="xsq")
                nc.scalar.activation(xsq[:], xn_rt[:], Act.Square)
                x2 = wp.tile([128, 1], f32, tag="x2")
                nc.vector.tensor_reduce(x2[:], xsq[:], Ax.X, Alu.add)

                # global idx per candidate = local + chunkbase
                cli_f = wp.tile([128, NCAND], f32, tag="clif")
                nc.vector.tensor_copy(cli_f[:], cand_li[rt][:])
                gidx_f = wp.tile([128, NCAND], f32, tag="gidxf")
                nc.vector.tensor_tensor(gidx_f[:], cli_f[:], cb_sb[:], Alu.add)

                # top-16 values (need 9)
                t8 = wp.tile([128, 8], f32, tag="t8")
                nc.vector.max(t8[:], cand_v[rt][:])
                cv2 = wp.tile([128, NCAND], f32, tag="cv2")
                nc.vector.match_replace(cv2[:], t8[:], cand_v[rt][:], -3.0e38)
                t8b = wp.tile([128, 8], f32, tag="t8b")
                nc.vector.max(t8b[:], cv2[:])
                v9 = wp.tile([128, KNN], f32, tag="v9")
                nc.vector.tensor_copy(v9[:, 0:8], t8[:])
                nc.vector.tensor_copy(v9[:, 8:9], t8b[:, 0:1])

                # d9 = sqrt(x2 - s)
                nc.scalar.activation(d9, v9[:], Act.Sqrt, bias=x2[:], scale=-1.0)

                # winner global indices: (cand_v == v9_j) * gidx -> reduce max
                gi9 = wp.tile([128, KNN], f32, tag="gi9")
                eqm = wp.tile([128, NCAND], f32, tag="eqm")
                for j in range(KNN):
                    nc.vector.scalar_tensor_tensor(
                        eqm[:], cand_v[rt][:], v9[:, j:j + 1], gidx_f[:],
                        Alu.is_equal, Alu.mult)
                    nc.vector.tensor_reduce(gi9[:, j:j + 1], eqm[:], Ax.X, Alu.max)

                # indices -> int16, shuffle into dma_gather layout
                gi16 = wp.tile([128, KNN], i16, tag="gi16")
                nc.vector.tensor_copy(gi16[:], gi9[:])
                scr = dramp.tile([RT, 16, KNN], i16, name=f"scr{rt}")
                nc.gpsimd.dma_start(scr[:], gi16[:])
                idxs_sb = wp.tile([128, RT * KNN], i16, tag="idxs")
                src = scr[:].rearrange("g c j -> c j g")
                for g2 in range(8):
                    nc.gpsimd.dma_start(idxs_sb[g2 * 16:(g2 + 1) * 16, :], src)

                # weights w = 1/(9d)
                rec = wp.tile([128, KNN], f32, tag="rec")
                nc.vector.reciprocal(rec[:], d9)
                wneg = wp.tile([128, KNN], f32, tag="wneg")
                nc.vector.tensor_scalar(wneg[:], rec[:], -1.0 / KNN, None, Alu.mult)
                s1s = wp.tile([128, 1], f32, tag="s1s")
                nc.vector.tensor_reduce(s1s[:], wneg[:], Ax.X, Alu.add)

                # gradient: g = -(x*sum(wneg) + sum_j wneg_j m_j) -> use
                # acc = x*s1s + sum_j wneg_j m_j ; g = -acc (fold into Abs)
                acc = [wp.tile([128, D], f32, tag="acca", name="acca"),
                       wp.tile([128, D], f32, tag="accb", name="accb")]
                # s1s = sum(wneg) < 0; x-term needs +x*sum(w) -> multiply by -1
                nc.vector.tensor_scalar(acc[0][:], xn_rt[:], s1s[:], -1.0,
                                        Alu.mult, Alu.mult)
                cur = 0
                # gather 9 bank vectors per row in groups of GG, accumulate
                for g in range(KNN // GG):
                    gm = gmp.tile([128, GG, D], f32, tag="gm")
                    nc.gpsimd.dma_gather(
                        gm[:], bank_d[:],
                        idxs_sb[:, g * 16 * GG // 16 * 8:0] if False else
                        idxs_sb[:, g * GG * 8:(g + 1) * GG * 8],
                        num_idxs=128 * GG, num_idxs_reg=128 * GG, elem_size=D)
                    for jj in range(GG):
                        j = g * GG + jj
                        nxt = 1 - cur
                        nc.vector.scalar_tensor_tensor(
                            acc[nxt][:], gm[:, jj, :], wneg[:, j:j + 1], acc[cur][:],
                            Alu.mult, Alu.add)
                        cur = nxt

                # influence = |g| * iw  (|-acc| == |acc|)
                absg = wp.tile([128, D], f32, tag="absg")
                nc.scalar.activation(absg[:], acc[cur][:], Act.Abs)
                nc.vector.tensor_tensor(inf_sb[rt][:], absg[:], iw_sb[:], Alu.mult)
                nc.sync.dma_start(inf_d[rt * 128:(rt + 1) * 128, :], inf_sb[rt][:])

                # influence row stats (unbiased std over D)
                isum = wp.tile([128, 1], f32, tag="isum")
                nc.vector.tensor_reduce(isum[:], inf_sb[rt][:], Ax.X, Alu.add)
                nc.vector.tensor_scalar(im_all[:, rt:rt + 1], isum[:], 1.0 / D,
                                        None, Alu.mult)
                icent = wp.tile([128, D], f32, tag="icent")
                nc.vector.tensor_scalar(icent[:], inf_sb[rt][:],
                                        im_all[:, rt:rt + 1], None, Alu.subtract)
                iss = wp.tile([128, 1], f32, tag="iss")
                sq_scr = wp.tile([128, D], f32, tag="sqscr")
                nc.vector.scalar_tensor_tensor(sq_scr[:], icent[:], 1.0, icent[:],
                                               Alu.mult, Alu.mult, accum_out=iss[:])
                istd = wp.tile([128, 1], f32, tag="istd")
                nc.scalar.activation(istd[:], iss[:], Act.Sqrt, scale=1.0 / (D - 1))
                istde = wp.tile([128, 1], f32, tag="istde")
                nc.vector.tensor_scalar(istde[:], istd[:], EPS, None, Alu.add)
                nc.vector.reciprocal(ii_all[:, rt:rt + 1], istde[:])

                # distance row-mean and square into dstat cols
                dsum = wp.tile([128, 1], f32, tag="dsum")
                nc.vector.tensor_reduce(dsum[:], d9, Ax.X, Alu.add)
                nc.vector.tensor_scalar(dstat[:, rt:rt + 1], dsum[:], 1.0 / KNN,
                                        None, Alu.mult)
                nc.vector.tensor_scalar(dstat[:, RT + rt:RT + rt + 1],
                                        dstat[:, rt:rt + 1],
                                        dstat[:, rt:rt + 1], None, Alu.mult)
                nc.sync.dma_start(dst_d[rt * 128:(rt + 1) * 128, :], d9)

            # ---- phase 3: global distance stats via matmul + AllReduce ----
            pst = psp.tile([128, 2 * RT], f32, tag="ps0", name="pst")
            nc.tensor.matmul(pst[0:1, :], ones_sb[:], dstat[:], start=True,
                             stop=True)
            colsum = wp.tile([128, 2 * RT], f32, tag="colsum")
            nc.vector.tensor_copy(colsum[0:1, :], pst[0:1, :])
            part = wp.tile([128, 8], f32, tag="part")
            nc.vector.memset(part[0:1, :], 0.0)
            nc.vector.tensor_reduce(part[0:1, 0:1], colsum[0:1, 0:RT], Ax.X,
                                    Alu.add)
            nc.vector.tensor_reduce(part[0:1, 1:2], colsum[0:1, RT:2 * RT], Ax.X,
                                    Alu.add)

            cc_in = dramp.tile([1, 8], f32)
            cc_out = dramp.tile([1, 8], f32)
            nc.gpsimd.dma_start(cc_in[:], part[0:1, :])
            nc.gpsimd.collective_compute(
                "AllReduce", mybir.AluOpType.add,
                replica_groups=[list(range(NCORES))],
                ins=[cc_in[:].opt()], outs=[cc_out[:].opt()])
            gst0 = wp.tile([1, 8], f32, tag="gst0")
            nc.gpsimd.dma_start(gst0[:], cc_out[:])
            gst = wp.tile([128, 8], f32, tag="gst")
            nc.gpsimd.partition_broadcast(gst[:], gst0[:])

            # gmean = S1/R ; var = D*(S2 - S1^2/R)/(R*D-1)
            gmean = wp.tile([128, 1], f32, tag="gmean")
            nc.vector.tensor_scalar(gmean[:], gst[:, 0:1], 1.0 / ROWS, None,
                                    Alu.mult)
            t1 = wp.tile([128, 1], f32, tag="t1")
            nc.vector.tensor_scalar(t1[:], gst[:, 0:1], gst[:, 0:1], 1.0 / ROWS,
                                    Alu.mult, Alu.mult)
            ssd = wp.tile([128, 1], f32, tag="ssd")
            nc.vector.tensor_tensor(ssd[:], gst[:, 1:2], t1[:], Alu.subtract)
            gstd = wp.tile([128, 1], f32, tag="gstd")
            nc.scalar.activation(gstd[:], ssd[:], Act.Sqrt,
                                 scale=float(D) / (ROWS * D - 1))
            gstde = wp.tile([128, 1], f32, tag="gstde")
            nc.vector.tensor_scalar(gstde[:], gstd[:], EPS, None, Alu.add)
            ginv = wp.tile([128, 1], f32, tag="ginv")
            nc.vector.reciprocal(ginv[:], gstde[:])
            dwg = wp.tile([128, 1], f32, tag="dwg")
            nc.vector.tensor_tensor(dwg[:], ginv[:], dw_sb[:], Alu.mult)

            # ---- phase 4: noise_std ----
            for rt in range(RT):
                crow = wp.tile([128, 1], f32, tag="crow")
                nc.vector.scalar_tensor_tensor(crow[:], dstat[:, rt:rt + 1],
                                               gmean[:], dwg[:],
                                               Alu.subtract, Alu.mult)
                mb = wp.tile([128, 1], f32, tag="mb")
                nc.vector.tensor_tensor(mb[:], im_all[:, rt:rt + 1],
                                        ii_all[:, rt:rt + 1], Alu.mult)
                bias = wp.tile([128, 1], f32, tag="bias")
                nc.vector.tensor_tensor(bias[:], crow[:], mb[:], Alu.subtract)
                sig = wp.tile([128, D], f32, tag="sig")
                nc.scalar.activation(sig[:], inf_sb[rt][:], Act.Sigmoid,
                                     bias=bias[:], scale=ii_all[:, rt:rt + 1])
                noi = wp.tile([128, D], f32, tag="noi")
                nc.vector.tensor_scalar(noi[:], sig[:], MAX_STD - MIN_STD, MIN_STD,
                                        Alu.mult, Alu.add)
                nc.sync.dma_start(noi_d[rt * 128:(rt + 1) * 128, :], noi[:])

    nc.compile()
    return nc


def _prep_inputs(features, memory_bank, influence_weight, distance_weight):
    x = np.ascontiguousarray(features.reshape(ROWS, D).astype(np.float32))
    m = np.ascontiguousarray(memory_bank.astype(np.float32))
    m2 = np.einsum("md,md->m", m, m).astype(np.float32)
    mta = np.empty((KD, M), np.float32)
    mta[:D] = m.T
    mta[D] = -m2
    mta = np.ascontiguousarray(mta)
    iw_rep = np.ascontiguousarray(
        np.broadcast_to(influence_weight.astype(np.float32), (128, D)))
    dw_rep = np.full((128, 1), np.float32(np.asarray(distance_weight).reshape(-1)[0]),
                     np.float32)
    cbase = np.ascontiguousarray(
        np.broadcast_to((np.arange(NCAND, dtype=np.float32) // 8) * CH,
                        (128, NCAND))).astype(np.float32)
    in_maps = []
    for c in range(NCORES):
        xs = x[c * RPC:(c + 1) * RPC]
        xa = np.empty((KD, RPC), np.float32)
        xa[:D] = 2.0 * xs.T
        xa[D] = 1.0
        in_maps.append({
            "xa": np.ascontiguousarray(xa),
            "xnat": np.ascontiguousarray(xs),
            "mta": mta,
            "bank": m,
            "iw": iw_rep,
            "dw": dw_rep,
            "cbase": cbase,
        })
    return in_maps


def kernel(features, memory_bank, influence_weight, distance_weight,
           _want_trace=False):
    from concourse.bass_utils import run_bass_kernel_spmd

    if "nc" not in _cache:
        _cache["nc"] = _build_nc()
    nc = _cache["nc"]

    in_maps = _prep_inputs(np.asarray(features), np.asarray(memory_bank),
                           np.asarray(influence_weight),
                           np.asarray(distance_weight))
    res = run_bass_kernel_spmd(nc, in_maps, core_ids=list(range(NCORES)),
                               trace=_want_trace)
    outs = res.results
    influence = np.concatenate([o["influence"] for o in outs], 0).reshape(B, N, D)
    noise = np.concatenate([o["noise"] for o in outs], 0).reshape(B, N, D)
    dist = np.concatenate([o["dist"] for o in outs], 0).reshape(B, N, KNN)
    if _want_trace:
        _cache["last_result"] = res
    return influence, noise, dist


# revision 7
# speedup vs baseline: 2.5170x; 2.5170x over previous
"""Trainium2 Bass kernel for nn_AdaptiveNoisingModule (retrieval_knn).

Strategy (8 NeuronCores, data-parallel over B*N rows):
  - each core owns 1024 of the 8192 query rows; memory bank replicated
  - scores s = 2*x.m - |m|^2 computed as one augmented fp32 matmul
    (top-9 smallest distances == top-9 largest s); bank streamed once,
    chunk-of-512 at a time, all 8 row-tiles share each bank chunk (PSUM-resident)
  - streaming top-k: per chunk, VectorE max/max_index reduce 512 cols -> 8
    candidates; 256 candidates per row merged at the end via
    max/match_replace/max + scalar_tensor_tensor(is_equal)*idx extraction
  - dma_gather fetches the 9 neighbor bank vectors per row for the exact
    gradient; influence/noise_std computed on-chip
  - one tiny AllReduce (8 cores) for the global distance-signal mean/std
"""

import numpy as np

B, N, D, M = 8, 1024, 1024, 16384
KNN = 9
NCORES = 8
ROWS = B * N                  # 8192
RPC = ROWS // NCORES          # 1024 rows per core
RT = RPC // 128               # 8 row-tiles of 128
CH = 512                      # bank chunk (1 PSUM bank of fp32)
NCH = M // CH                 # 32 chunks
KD = D + 2                    # augmented rows: -m2_hi, -m2_lo (fp16 split)
NKS = (KD + 127) // 128       # 9 k-steps (last has 1 partition)
NCAND = NCH * 8               # 256 candidates per row
GG = 3                        # gather group: neighbors per dma_gather
MIN_STD, MAX_STD, EPS = 0.01, 0.5, 1e-8

_cache = {}


def _build_nc():
    import concourse.bass as bass
    import concourse.bacc as bacc
    import concourse.mybir as mybir
    import concourse.tile as tile

    f32 = mybir.dt.float32
    f32r = mybir.dt.float32r
    f16 = mybir.dt.float16
    i16 = mybir.dt.int16
    u16 = mybir.dt.uint16
    Alu = mybir.AluOpType
    Act = mybir.ActivationFunctionType
    Ax = mybir.AxisListType

    nc = bacc.Bacc("TRN2", target_bir_lowering=False, debug=False,
                   num_devices=NCORES)

    # ---- I/O ----
    xa_d = nc.dram_tensor("xa", [KD, RPC], f16, kind="ExternalInput")
    xn_d = nc.dram_tensor("xnat", [RPC, D], f32, kind="ExternalInput")
    mta_d = nc.dram_tensor("mta", [KD, M], f16, kind="ExternalInput")
    bank_d = nc.dram_tensor("bank", [M, D], f32, kind="ExternalInput")
    iw_d = nc.dram_tensor("iw", [128, D], f32, kind="ExternalInput")
    dw_d = nc.dram_tensor("dw", [128, 1], f32, kind="ExternalInput")
    cb_d = nc.dram_tensor("cbase", [128, NCAND], f32, kind="ExternalInput")

    inf_d = nc.dram_tensor("influence", [RPC, D], f32, kind="ExternalOutput")
    noi_d = nc.dram_tensor("noise", [RPC, D], f32, kind="ExternalOutput")
    dst_d = nc.dram_tensor("dist", [RPC, KNN], f32, kind="ExternalOutput")

    with tile.TileContext(nc) as tc:
        with (
            tc.tile_pool(name="const", bufs=1) as cp,
            tc.tile_pool(name="mt", bufs=2) as mtp,
            tc.tile_pool(name="sch", bufs=4) as schp,
            tc.tile_pool(name="cand", bufs=1) as candp,
            tc.tile_pool(name="work", bufs=1) as wp,
            tc.tile_pool(name="gm", bufs=2) as gmp,
            tc.tile_pool(name="ps", bufs=1, space="PSUM") as psp,
            tc.tile_pool(name="dram", bufs=1, space="DRAM") as dramp,
        ):
            # ---- constants / resident tensors ----
            xa_sb = []
            for k in range(NKS):
                p = min(128, KD - k * 128)
                t = cp.tile([p, RPC], f16, tag=f"xa{k}", name=f"xa{k}")
                nc.sync.dma_start(t[:], xa_d[k * 128:k * 128 + p, :])
                xa_sb.append(t)
            iw_sb = cp.tile([128, D], f32, tag="iw")
            nc.sync.dma_start(iw_sb[:], iw_d[:])
            dw_sb = cp.tile([128, 1], f32, tag="dw")
            nc.sync.dma_start(dw_sb[:], dw_d[:])
            cb_sb = cp.tile([128, NCAND], f32, tag="cb")
            nc.sync.dma_start(cb_sb[:], cb_d[:])
            ones_sb = cp.tile([128, 1], f32, tag="ones")
            nc.vector.memset(ones_sb[:], 1.0)

            # per-row-tile stores that live across phases
            cand_v = [candp.tile([128, NCAND], f32, tag=f"cv{rt}", name=f"cv{rt}")
                      for rt in range(RT)]
            cand_li = [candp.tile([128, NCAND], u16, tag=f"cl{rt}", name=f"cl{rt}")
                       for rt in range(RT)]
            inf_sb = [candp.tile([128, D], f32, tag=f"inf{rt}", name=f"inf{rt}")
                      for rt in range(RT)]
            d9_all = candp.tile([128, RT * KNN], f32, tag="d9all")
            im_all = candp.tile([128, RT], f32, tag="imall")
            ii_all = candp.tile([128, RT], f32, tag="iiall")
            dstat = candp.tile([128, 2 * RT], f32, tag="dstat")

            # ---- phase 1: streaming matmul + per-chunk top-8 ----
            for n in range(NCH):
                mts = []
                for k in range(NKS):
                    p = min(128, KD - k * 128)
                    mt = mtp.tile([p, CH], f16, tag=f"mt{k}", name=f"mt{k}")
                    nc.sync.dma_start(mt[:], mta_d[k * 128:k * 128 + p,
                                                   n * CH:(n + 1) * CH])
                    mts.append(mt)
                for rt in range(RT):
                    ps = psp.tile([128, CH], f32, tag=f"ps{rt}", name=f"ps{rt}")
                    for k in range(NKS):
                        nc.tensor.matmul(
                            ps[:], xa_sb[k][:, rt * 128:(rt + 1) * 128], mts[k][:],
                            start=(k == 0), stop=(k == NKS - 1))
                    sch = schp.tile([128, CH], f32, tag="sch")
                    nc.scalar.copy(sch[:], ps[:])
                    nc.vector.max(cand_v[rt][:, n * 8:(n + 1) * 8], sch[:])
                    nc.vector.max_index(cand_li[rt][:, n * 8:(n + 1) * 8],
                                        cand_v[rt][:, n * 8:(n + 1) * 8], sch[:])

            # ---- phase 2: per row-tile merge, gather, gradient ----
            for rt in range(RT):
                d9 = d9_all[:, rt * KNN:(rt + 1) * KNN]

                # x natural + x2
                xn_rt = wp.tile([128, D], f32, tag="xnrt")
                nc.sync.dma_start(xn_rt[:], xn_d[rt * 128:(rt + 1) * 128, :])
                xsq = wp.tile([128, D], f32, tag="xsq")
                nc.scalar.activation(xsq[:], xn_rt[:], Act.Square)
                x2 = wp.tile([128, 1], f32, tag="x2")
                nc.vector.tensor_reduce(x2[:], xsq[:], Ax.X, Alu.add)

                # global idx per candidate = local + chunkbase
                cli_f = wp.tile([128, NCAND], f32, tag="clif")
                nc.vector.tensor_copy(cli_f[:], cand_li[rt][:])
                gidx_f = wp.tile([128, NCAND], f32, tag="gidxf")
                nc.vector.tensor_tensor(gidx_f[:], cli_f[:], cb_sb[:], Alu.add)

                # top-16 values (need 9)
                t8 = wp.tile([128, 8], f32, tag="t8")
                nc.vector.max(t8[:], cand_v[rt][:])
                cv2 = wp.tile([128, NCAND], f32, tag="cv2")
                nc.vector.match_replace(cv2[:], t8[:], cand_v[rt][:], -3.0e38)
                t8b = wp.tile([128, 8], f32, tag="t8b")
                nc.vector.max(t8b[:], cv2[:])
                v9 = wp.tile([128, KNN], f32, tag="v9")
                nc.vector.tensor_copy(v9[:, 0:8], t8[:])
                nc.vector.tensor_copy(v9[:, 8:9], t8b[:, 0:1])

                # d9 = sqrt(x2 - s)
                nc.scalar.activation(d9, v9[:], Act.Sqrt, bias=x2[:], scale=-1.0)

                # winner global indices: (cand_v == v9_j) * gidx -> reduce max
                gi9 = wp.tile([128, KNN], f32, tag="gi9")
                eqm = wp.tile([128, NCAND], f32, tag="eqm")
                for j in range(KNN):
                    nc.vector.scalar_tensor_tensor(
                        eqm[:], cand_v[rt][:], v9[:, j:j + 1], gidx_f[:],
                        Alu.is_equal, Alu.mult)
                    nc.vector.tensor_reduce(gi9[:, j:j + 1], eqm[:], Ax.X, Alu.max)

                # indices -> int16, shuffle into dma_gather layout
                gi16 = wp.tile([128, KNN], i16, tag="gi16")
                nc.vector.tensor_copy(gi16[:], gi9[:])
                scr = dramp.tile([RT, 16, KNN], i16, name=f"scr{rt}")
                nc.gpsimd.dma_start(scr[:], gi16[:])
                idxs_sb = wp.tile([128, RT * KNN], i16, tag="idxs")
                src = scr[:].rearrange("g c j -> c j g")
                for g2 in range(8):
                    nc.gpsimd.dma_start(idxs_sb[g2 * 16:(g2 + 1) * 16, :], src)

                # weights w = 1/(9d)
                rec = wp.tile([128, KNN], f32, tag="rec")
                nc.vector.reciprocal(rec[:], d9)
                wneg = wp.tile([128, KNN], f32, tag="wneg")
                nc.vector.tensor_scalar(wneg[:], rec[:], -1.0 / KNN, None, Alu.mult)
                s1s = wp.tile([128, 1], f32, tag="s1s")
                nc.vector.tensor_reduce(s1s[:], wneg[:], Ax.X, Alu.add)

                # gradient: g = -(x*sum(wneg) + sum_j wneg_j m_j) -> use
                # acc = x*s1s + sum_j wneg_j m_j ; g = -acc (fold into Abs)
                acc = [wp.tile([128, D], f32, tag="acca", name="acca"),
                       wp.tile([128, D], f32, tag="accb", name="accb")]
                # s1s = sum(wneg) < 0; x-term needs +x*sum(w) -> multiply by -1
                nc.vector.tensor_scalar(acc[0][:], xn_rt[:], s1s[:], -1.0,
                                        Alu.mult, Alu.mult)
                cur = 0
                # gather 9 bank vectors per row in groups of GG, accumulate
                for g in range(KNN // GG):
                    gm = gmp.tile([128, GG, D], f32, tag="gm")
                    nc.gpsimd.dma_gather(
                        gm[:], bank_d[:],
                        idxs_sb[:, g * 16 * GG // 16 * 8:0] if False else
                        idxs_sb[:, g * GG * 8:(g + 1) * GG * 8],
                        num_idxs=128 * GG, num_idxs_reg=128 * GG, elem_size=D)
                    for jj in range(GG):
                        j = g * GG + jj
                        nxt = 1 - cur
                        nc.vector.scalar_tensor_tensor(
                            acc[nxt][:], gm[:, jj, :], wneg[:, j:j + 1], acc[cur][:],
                            Alu.mult, Alu.add)
                        cur = nxt

                # influence = |g| * iw  (|-acc| == |acc|)
                absg = wp.tile([128, D], f32, tag="absg")
                nc.scalar.activation(absg[:], acc[cur][:], Act.Abs)
                nc.vector.tensor_tensor(inf_sb[rt][:], absg[:], iw_sb[:], Alu.mult)
                nc.sync.dma_start(inf_d[rt * 128:(rt + 1) * 128, :], inf_sb[rt][:])

                # influence row stats (unbiased std over D)
                isum = wp.tile([128, 1], f32, tag="isum")
                nc.vector.tensor_reduce(isum[:], inf_sb[rt][:], Ax.X, Alu.add)
                nc.vector.tensor_scalar(im_all[:, rt:rt + 1], isum[:], 1.0 / D,
                                        None, Alu.mult)
                icent = wp.tile([128, D], f32, tag="icent")
                nc.vector.tensor_scalar(icent[:], inf_sb[rt][:],
                                        im_all[:, rt:rt + 1], None, Alu.subtract)
                iss = wp.tile([128, 1], f32, tag="iss")
                sq_scr = wp.tile([128, D], f32, tag="sqscr")
                nc.vector.scalar_tensor_tensor(sq_scr[:], icent[:], 1.0, icent[:],
                                               Alu.mult, Alu.mult, accum_out=iss[:])
                istd = wp.tile([128, 1], f32, tag="istd")
                nc.scalar.activation(istd[:], iss[:], Act.Sqrt, scale=1.0 / (D - 1))
                istde = wp.tile([128, 1], f32, tag="istde")
                nc.vector.tensor_scalar(istde[:], istd[:], EPS, None, Alu.add)
                nc.vector.reciprocal(ii_all[:, rt:rt + 1], istde[:])

                # distance row-mean and square into dstat cols
                dsum = wp.tile([128, 1], f32, tag="dsum")
                nc.vector.tensor_reduce(dsum[:], d9, Ax.X, Alu.add)
                nc.vector.tensor_scalar(dstat[:, rt:rt + 1], dsum[:], 1.0 / KNN,
                                        None, Alu.mult)
                nc.vector.tensor_scalar(dstat[:, RT + rt:RT + rt + 1],
                                        dstat[:, rt:rt + 1],
                                        dstat[:, rt:rt + 1], None, Alu.mult)
                nc.sync.dma_start(dst_d[rt * 128:(rt + 1) * 128, :], d9)

            # ---- phase 3: global distance stats via matmul + AllReduce ----
            pst = psp.tile([128, 2 * RT], f32, tag="ps0", name="pst")
            nc.tensor.matmul(pst[0:1, :], ones_sb[:], dstat[:], start=True,
                             stop=True)
            colsum = wp.tile([128, 2 * RT], f32, tag="colsum")
            nc.vector.tensor_copy(colsum[0:1, :], pst[0:1, :])
            part = wp.tile([128, 8], f32, tag="part")
            nc.vector.memset(part[0:1, :], 0.0)
            nc.vector.tensor_reduce(part[0:1, 0:1], colsum[0:1, 0:RT], Ax.X,
                                    Alu.add)
            nc.vector.tensor_reduce(part[0:1, 1:2], colsum[0:1, RT:2 * RT], Ax.X,
                                    Alu.add)

            cc_in = dramp.tile([1, 8], f32)
            cc_out = dramp.tile([1, 8], f32)
            nc.gpsimd.dma_start(cc_in[:], part[0:1, :])
            nc.gpsimd.collective_compute(
                "AllReduce", mybir.AluOpType.add,
                replica_groups=[list(range(NCORES))],
                ins=[cc_in[:].opt()], outs=[cc_out[:].opt()])
            gst0 = wp.tile([1, 8], f32, tag="gst0")
            nc.gpsimd.dma_start(gst0[:], cc_out[:])
            gst = wp.tile([128, 8], f32, tag="gst")
            nc.gpsimd.partition_broadcast(gst[:], gst0[:])

            # gmean = S1/R ; var = D*(S2 - S1^2/R)/(R*D-1)
            gmean = wp.tile([128, 1], f32, tag="gmean")
            nc.vector.tensor_scalar(gmean[:], gst[:, 0:1], 1.0 / ROWS, None,
                                    Alu.mult)
            t1 = wp.tile([128, 1], f32, tag="t1")
            nc.vector.tensor_scalar(t1[:], gst[:, 0:1], gst[:, 0:1], 1.0 / ROWS,
                                    Alu.mult, Alu.mult)
            ssd = wp.tile([128, 1], f32, tag="ssd")
            nc.vector.tensor_tensor(ssd[:], gst[:, 1:2], t1[:], Alu.subtract)
            gstd = wp.tile([128, 1], f32, tag="gstd")
            nc.scalar.activation(gstd[:], ssd[:], Act.Sqrt,
                                 scale=float(D) / (ROWS * D - 1))
            gstde = wp.tile([128, 1], f32, tag="gstde")
            nc.vector.tensor_scalar(gstde[:], gstd[:], EPS, None, Alu.add)
            ginv = wp.tile([128, 1], f32, tag="ginv")
            nc.vector.reciprocal(ginv[:], gstde[:])
            dwg = wp.tile([128, 1], f32, tag="dwg")
            nc.vector.tensor_tensor(dwg[:], ginv[:], dw_sb[:], Alu.mult)

            # ---- phase 4: noise_std ----
            for rt in range(RT):
                crow = wp.tile([128, 1], f32, tag="crow")
                nc.vector.scalar_tensor_tensor(crow[:], dstat[:, rt:rt + 1],
                                               gmean[:], dwg[:],
                                               Alu.subtract, Alu.mult)
                mb = wp.tile([128, 1], f32, tag="mb")
                nc.vector.tensor_tensor(mb[:], im_all[:, rt:rt + 1],
                                        ii_all[:, rt:rt + 1], Alu.mult)
                bias = wp.tile([128, 1], f32, tag="bias")
                nc.vector.tensor_tensor(bias[:], crow[:], mb[:], Alu.subtract)
                sig = wp.tile([128, D], f32, tag="sig")
                nc.scalar.activation(sig[:], inf_sb[rt][:], Act.Sigmoid,
                                     bias=bias[:], scale=ii_all[:, rt:rt + 1])
                noi = wp.tile([128, D], f32, tag="noi")
                nc.vector.tensor_scalar(noi[:], sig[:], MAX_STD - MIN_STD, MIN_STD,
                                        Alu.mult, Alu.add)
                nc.sync.dma_start(noi_d[rt * 128:(rt + 1) * 128, :], noi[:])

    nc.compile()
    return nc


def _prep_inputs(features, memory_bank, influence_weight, distance_weight):
    x = np.ascontiguousarray(features.reshape(ROWS, D).astype(np.float32))
    m = np.ascontiguousarray(memory_bank.astype(np.float32))
    m2_64 = np.einsum("md,md->m", m.astype(np.float64), m.astype(np.float64))
    m2h = m2_64.astype(np.float16)
    m2l = (m2_64 - m2h.astype(np.float64)).astype(np.float16)
    mta = np.empty((KD, M), np.float16)
    mta[:D] = m.T.astype(np.float16)
    mta[D] = -m2h
    mta[D + 1] = -m2l
    mta = np.ascontiguousarray(mta)
    iw_rep = np.ascontiguousarray(
        np.broadcast_to(influence_weight.astype(np.float32), (128, D)))
    dw_rep = np.full((128, 1), np.float32(np.asarray(distance_weight).reshape(-1)[0]),
                     np.float32)
    cbase = np.ascontiguousarray(
        np.broadcast_to((np.arange(NCAND, dtype=np.float32) // 8) * CH,
                        (128, NCAND))).astype(np.float32)
    in_maps = []
    for c in range(NCORES):
        xs = x[c * RPC:(c + 1) * RPC]
        xa = np.empty((KD, RPC), np.float16)
        xa[:D] = (2.0 * xs.T).astype(np.float16)
        xa[D] = 1.0
        xa[D + 1] = 1.0
        in_maps.append({
            "xa": np.ascontiguousarray(xa),
            "xnat": np.ascontiguousarray(xs),
            "mta": mta,
            "bank": m,
            "iw": iw_rep,
            "dw": dw_rep,
            "cbase": cbase,
        })
    return in_maps


def kernel(features, memory_bank, influence_weight, distance_weight,
           _want_trace=False):
    from concourse.bass_utils import run_bass_kernel_spmd

    if "nc" not in _cache:
        _cache["nc"] = _build_nc()
    nc = _cache["nc"]

    in_maps = _prep_inputs(np.asarray(features), np.asarray(memory_bank),
                           np.asarray(influence_weight),
                           np.asarray(distance_weight))
    res = run_bass_kernel_spmd(nc, in_maps, core_ids=list(range(NCORES)),
                               trace=_want_trace)
    outs = res.results
    influence = np.concatenate([o["influence"] for o in outs], 0).reshape(B, N, D)
    noise = np.concatenate([o["noise"] for o in outs], 0).reshape(B, N, D)
    dist = np.concatenate([o["dist"] for o in outs], 0).reshape(B, N, KNN)
    if _want_trace:
        _cache["last_result"] = res
    return influence, noise, dist


# revision 8
# speedup vs baseline: 2.7385x; 1.0880x over previous
"""Trainium2 Bass kernel for nn_AdaptiveNoisingModule (retrieval_knn).

Strategy (8 NeuronCores, data-parallel over B*N rows):
  - each core owns 1024 of the 8192 query rows; memory bank replicated
  - scores s = 2*x.m - |m|^2 computed as one augmented fp32 matmul
    (top-9 smallest distances == top-9 largest s); bank streamed once,
    chunk-of-512 at a time, all 8 row-tiles share each bank chunk (PSUM-resident)
  - streaming top-k: per chunk, VectorE max/max_index reduce 512 cols -> 8
    candidates; 256 candidates per row merged at the end via
    max/match_replace/max + scalar_tensor_tensor(is_equal)*idx extraction
  - dma_gather fetches the 9 neighbor bank vectors per row for the exact
    gradient; influence/noise_std computed on-chip
  - one tiny AllReduce (8 cores) for the global distance-signal mean/std
"""

import numpy as np

B, N, D, M = 8, 1024, 1024, 16384
KNN = 9
NCORES = 8
ROWS = B * N                  # 8192
RPC = ROWS // NCORES          # 1024 rows per core
RT = RPC // 128               # 8 row-tiles of 128
CH = 512                      # bank chunk (1 PSUM bank of fp32)
NCH = M // CH                 # 32 chunks
KD = D + 2                    # augmented rows: -m2_hi, -m2_lo (fp16 split)
NKS = (KD + 127) // 128       # 9 k-steps (last has 1 partition)
NCAND = NCH * 8               # 256 candidates per row
GG = 3                        # gather group: neighbors per dma_gather
MIN_STD, MAX_STD, EPS = 0.01, 0.5, 1e-8

_cache = {}


def _build_nc():
    import concourse.bass as bass
    import concourse.bacc as bacc
    import concourse.mybir as mybir
    import concourse.tile as tile

    f32 = mybir.dt.float32
    f32r = mybir.dt.float32r
    f16 = mybir.dt.float16
    i16 = mybir.dt.int16
    u16 = mybir.dt.uint16
    Alu = mybir.AluOpType
    Act = mybir.ActivationFunctionType
    Ax = mybir.AxisListType

    nc = bacc.Bacc("TRN2", target_bir_lowering=False, debug=False,
                   num_devices=NCORES)

    # ---- I/O ----
    xa_d = nc.dram_tensor("xa", [KD, RPC], f16, kind="ExternalInput")
    xn_d = nc.dram_tensor("xnat", [RPC, D], f32, kind="ExternalInput")
    mta_d = nc.dram_tensor("mta", [KD, M], f16, kind="ExternalInput")
    bank_d = nc.dram_tensor("bank", [M, D], f32, kind="ExternalInput")
    iw_d = nc.dram_tensor("iw", [128, D], f32, kind="ExternalInput")
    dw_d = nc.dram_tensor("dw", [128, 1], f32, kind="ExternalInput")
    cb_d = nc.dram_tensor("cbase", [128, NCAND], f32, kind="ExternalInput")

    inf_d = nc.dram_tensor("influence", [RPC, D], f32, kind="ExternalOutput")
    noi_d = nc.dram_tensor("noise", [RPC, D], f32, kind="ExternalOutput")
    dst_d = nc.dram_tensor("dist", [RPC, KNN], f32, kind="ExternalOutput")

    with tile.TileContext(nc) as tc:
        with (
            tc.tile_pool(name="const", bufs=1) as cp,
            tc.tile_pool(name="mt", bufs=2) as mtp,
            tc.tile_pool(name="sch", bufs=4) as schp,
            tc.tile_pool(name="cand", bufs=1) as candp,
            tc.tile_pool(name="work", bufs=1) as wp,
            tc.tile_pool(name="wa", bufs=2) as wpa,
            tc.tile_pool(name="gm", bufs=2) as gmp,
            tc.tile_pool(name="ps", bufs=1, space="PSUM") as psp,
            tc.tile_pool(name="dram", bufs=1, space="DRAM") as dramp,
        ):
            # ---- constants / resident tensors ----
            xa_sb = []
            for k in range(NKS):
                p = min(128, KD - k * 128)
                t = cp.tile([p, RPC], f16, tag=f"xa{k}", name=f"xa{k}")
                nc.sync.dma_start(t[:], xa_d[k * 128:k * 128 + p, :])
                xa_sb.append(t)
            iw_sb = cp.tile([128, D], f32, tag="iw")
            nc.sync.dma_start(iw_sb[:], iw_d[:])
            dw_sb = cp.tile([128, 1], f32, tag="dw")
            nc.sync.dma_start(dw_sb[:], dw_d[:])
            cb_sb = cp.tile([128, NCAND], f32, tag="cb")
            nc.sync.dma_start(cb_sb[:], cb_d[:])
            ones_sb = cp.tile([128, 1], f32, tag="ones")
            nc.vector.memset(ones_sb[:], 1.0)

            # per-row-tile stores that live across phases
            cand_v = [candp.tile([128, NCAND], f32, tag=f"cv{rt}", name=f"cv{rt}")
                      for rt in range(RT)]
            cand_li = [candp.tile([128, NCAND], u16, tag=f"cl{rt}", name=f"cl{rt}")
                       for rt in range(RT)]
            inf_sb = [candp.tile([128, D], f32, tag=f"inf{rt}", name=f"inf{rt}")
                      for rt in range(RT)]
            d9_all = candp.tile([128, RT * KNN], f32, tag="d9all")
            im_all = candp.tile([128, RT], f32, tag="imall")
            ii_all = candp.tile([128, RT], f32, tag="iiall")
            dstat = candp.tile([128, 2 * RT], f32, tag="dstat")
            wneg_all = candp.tile([128, RT * KNN], f32, tag="wnegall")
            s1s_all = candp.tile([128, RT], f32, tag="s1sall")
            idx_sb_all = [candp.tile([128, RT * KNN], i16, tag=f"idx{rt}",
                                     name=f"idx{rt}") for rt in range(RT)]

            # ---- phase 1: streaming matmul + per-chunk top-8 ----
            for n in range(NCH):
                mts = []
                for k in range(NKS):
                    p = min(128, KD - k * 128)
                    mt = mtp.tile([p, CH], f16, tag=f"mt{k}", name=f"mt{k}")
                    nc.sync.dma_start(mt[:], mta_d[k * 128:k * 128 + p,
                                                   n * CH:(n + 1) * CH])
                    mts.append(mt)
                for rt in range(RT):
                    ps = psp.tile([128, CH], f32, tag=f"ps{rt}", name=f"ps{rt}")
                    for k in range(NKS):
                        nc.tensor.matmul(
                            ps[:], xa_sb[k][:, rt * 128:(rt + 1) * 128], mts[k][:],
                            start=(k == 0), stop=(k == NKS - 1))
                    sch = schp.tile([128, CH], f32, tag="sch")
                    nc.scalar.copy(sch[:], ps[:])
                    nc.vector.max(cand_v[rt][:, n * 8:(n + 1) * 8], sch[:])
                    nc.vector.max_index(cand_li[rt][:, n * 8:(n + 1) * 8],
                                        cand_v[rt][:, n * 8:(n + 1) * 8], sch[:])

            # ---- phase 2a: per row-tile merge, d9, winner indices ----
            for rt in range(RT):
                d9 = d9_all[:, rt * KNN:(rt + 1) * KNN]

                xn_rt = wpa.tile([128, D], f32, tag="xnrt")
                nc.sync.dma_start(xn_rt[:], xn_d[rt * 128:(rt + 1) * 128, :])
                xsq = wpa.tile([128, D], f32, tag="xsq")
                nc.scalar.activation(xsq[:], xn_rt[:], Act.Square)
                x2 = wpa.tile([128, 1], f32, tag="x2")
                nc.vector.tensor_reduce(x2[:], xsq[:], Ax.X, Alu.add)

                # global idx per candidate = local + chunkbase
                cli_f = wpa.tile([128, NCAND], f32, tag="clif")
                nc.vector.tensor_copy(cli_f[:], cand_li[rt][:])
                gidx_f = wpa.tile([128, NCAND], f32, tag="gidxf")
                nc.vector.tensor_tensor(gidx_f[:], cli_f[:], cb_sb[:], Alu.add)

                # top-16 values (need 9)
                t8 = wpa.tile([128, 8], f32, tag="t8")
                nc.vector.max(t8[:], cand_v[rt][:])
                cv2 = wpa.tile([128, NCAND], f32, tag="cv2")
                nc.vector.match_replace(cv2[:], t8[:], cand_v[rt][:], -3.0e38)
                t8b = wpa.tile([128, 8], f32, tag="t8b")
                nc.vector.max(t8b[:], cv2[:])
                v9 = wpa.tile([128, KNN], f32, tag="v9")
                nc.vector.tensor_copy(v9[:, 0:8], t8[:])
                nc.vector.tensor_copy(v9[:, 8:9], t8b[:, 0:1])

                # d9 = sqrt(x2 - s)
                nc.scalar.activation(d9, v9[:], Act.Sqrt, bias=x2[:], scale=-1.0)

                # winner global indices: (cand_v == v9_j) * gidx -> reduce max
                gi9 = wpa.tile([128, KNN], f32, tag="gi9")
                eqm = wpa.tile([128, NCAND], f32, tag="eqm")
                for j in range(KNN):
                    nc.vector.scalar_tensor_tensor(
                        eqm[:], cand_v[rt][:], v9[:, j:j + 1], gidx_f[:],
                        Alu.is_equal, Alu.mult)
                    nc.vector.tensor_reduce(gi9[:, j:j + 1], eqm[:], Ax.X, Alu.max)

                # indices -> int16, shuffle into dma_gather layout (HWDGE)
                gi16 = wpa.tile([128, KNN], i16, tag="gi16")
                nc.vector.tensor_copy(gi16[:], gi9[:])
                scr = dramp.tile([RT, 16, KNN], i16, name=f"scr{rt}")
                nc.sync.dma_start(scr[:], gi16[:])
                idxs_sb = idx_sb_all[rt]
                src_ap = scr[:].rearrange("g c j -> c j g")
                for g2 in range(8):
                    eng = nc.sync if g2 % 2 == 0 else nc.scalar
                    eng.dma_start(idxs_sb[g2 * 16:(g2 + 1) * 16, :], src_ap)

                # weights w = 1/(9d)
                rec = wpa.tile([128, KNN], f32, tag="rec")
                nc.vector.reciprocal(rec[:], d9)
                nc.vector.tensor_scalar(wneg_all[:, rt * KNN:(rt + 1) * KNN],
                                        rec[:], -1.0 / KNN, None, Alu.mult)
                nc.vector.tensor_reduce(s1s_all[:, rt:rt + 1],
                                        wneg_all[:, rt * KNN:(rt + 1) * KNN],
                                        Ax.X, Alu.add)

                # distance row-mean and square into dstat cols
                dsum = wpa.tile([128, 1], f32, tag="dsum")
                nc.vector.tensor_reduce(dsum[:], d9, Ax.X, Alu.add)
                nc.vector.tensor_scalar(dstat[:, rt:rt + 1], dsum[:], 1.0 / KNN,
                                        None, Alu.mult)
                nc.vector.tensor_scalar(dstat[:, RT + rt:RT + rt + 1],
                                        dstat[:, rt:rt + 1],
                                        dstat[:, rt:rt + 1], None, Alu.mult)
                nc.sync.dma_start(dst_d[rt * 128:(rt + 1) * 128, :], d9)

            # ---- partial sums for global distance stats ----
            pst = psp.tile([128, 2 * RT], f32, tag="ps0", name="pst")
            nc.tensor.matmul(pst[0:1, :], ones_sb[:], dstat[:], start=True,
                             stop=True)
            colsum = wp.tile([128, 2 * RT], f32, tag="colsum")
            nc.vector.tensor_copy(colsum[0:1, :], pst[0:1, :])
            part = wp.tile([128, 8], f32, tag="part")
            nc.vector.memset(part[0:1, :], 0.0)
            nc.vector.tensor_reduce(part[0:1, 0:1], colsum[0:1, 0:RT], Ax.X,
                                    Alu.add)
            nc.vector.tensor_reduce(part[0:1, 1:2], colsum[0:1, RT:2 * RT], Ax.X,
                                    Alu.add)
            cc_in = dramp.tile([1, 8], f32)
            cc_out = dramp.tile([1, 8], f32)
            nc.sync.dma_start(cc_in[:], part[0:1, :])

            # ---- phase 2b: gather + gradient + influence + row stats ----
            for rt in range(RT):
                xn_rt2 = wpa.tile([128, D], f32, tag="xnrt2")
                nc.sync.dma_start(xn_rt2[:], xn_d[rt * 128:(rt + 1) * 128, :])
                wneg = wneg_all[:, rt * KNN:(rt + 1) * KNN]

                acc = [wp.tile([128, D], f32, tag="acca", name="acca"),
                       wp.tile([128, D], f32, tag="accb", name="accb")]
                # s1s = sum(wneg) < 0; x-term needs +x*sum(w) -> multiply by -1
                nc.vector.tensor_scalar(acc[0][:], xn_rt2[:],
                                        s1s_all[:, rt:rt + 1], -1.0,
                                        Alu.mult, Alu.mult)
                cur = 0
                for g in range(KNN // GG):
                    gm = gmp.tile([128, GG, D], f32, tag="gm")
                    nc.gpsimd.dma_gather(
                        gm[:], bank_d[:],
                        idx_sb_all[rt][:, g * GG * 8:(g + 1) * GG * 8],
                        num_idxs=128 * GG, num_idxs_reg=128 * GG, elem_size=D)
                    for jj in range(GG):
                        j = g * GG + jj
                        nxt = 1 - cur
                        nc.vector.scalar_tensor_tensor(
                            acc[nxt][:], gm[:, jj, :], wneg[:, j:j + 1],
                            acc[cur][:], Alu.mult, Alu.add)
                        cur = nxt

                # influence = |g| * iw  (|-acc| == |acc|)
                absg = wp.tile([128, D], f32, tag="absg")
                nc.scalar.activation(absg[:], acc[cur][:], Act.Abs)
                nc.vector.tensor_tensor(inf_sb[rt][:], absg[:], iw_sb[:], Alu.mult)
                nc.sync.dma_start(inf_d[rt * 128:(rt + 1) * 128, :], inf_sb[rt][:])

                # influence row stats (unbiased std over D)
                isum = wp.tile([128, 1], f32, tag="isum")
                nc.vector.tensor_reduce(isum[:], inf_sb[rt][:], Ax.X, Alu.add)
                nc.vector.tensor_scalar(im_all[:, rt:rt + 1], isum[:], 1.0 / D,
                                        None, Alu.mult)
                icent = wp.tile([128, D], f32, tag="icent")
                nc.vector.tensor_scalar(icent[:], inf_sb[rt][:],
                                        im_all[:, rt:rt + 1], None, Alu.subtract)
                iss = wp.tile([128, 1], f32, tag="iss")
                sq_scr = wp.tile([128, D], f32, tag="sqscr")
                nc.vector.scalar_tensor_tensor(sq_scr[:], icent[:], 1.0, icent[:],
                                               Alu.mult, Alu.mult, accum_out=iss[:])
                istd = wp.tile([128, 1], f32, tag="istd")
                nc.scalar.activation(istd[:], iss[:], Act.Sqrt, scale=1.0 / (D - 1))
                istde = wp.tile([128, 1], f32, tag="istde")
                nc.vector.tensor_scalar(istde[:], istd[:], EPS, None, Alu.add)
                nc.vector.reciprocal(ii_all[:, rt:rt + 1], istde[:])

            # ---- AllReduce (overlaps the tail of 2b) ----
            nc.gpsimd.collective_compute(
                "AllReduce", mybir.AluOpType.add,
                replica_groups=[list(range(NCORES))],
                ins=[cc_in[:].opt()], outs=[cc_out[:].opt()])
            gst0 = wp.tile([1, 8], f32, tag="gst0")
            nc.sync.dma_start(gst0[:], cc_out[:])
            gst = wp.tile([128, 8], f32, tag="gst")
            nc.gpsimd.partition_broadcast(gst[:], gst0[:])

            # gmean = S1/R ; var = D*(S2 - S1^2/R)/(R*D-1)
            gmean = wp.tile([128, 1], f32, tag="gmean")
            nc.vector.tensor_scalar(gmean[:], gst[:, 0:1], 1.0 / ROWS, None,
                                    Alu.mult)
            t1 = wp.tile([128, 1], f32, tag="t1")
            nc.vector.tensor_scalar(t1[:], gst[:, 0:1], gst[:, 0:1], 1.0 / ROWS,
                                    Alu.mult, Alu.mult)
            ssd = wp.tile([128, 1], f32, tag="ssd")
            nc.vector.tensor_tensor(ssd[:], gst[:, 1:2], t1[:], Alu.subtract)
            gstd = wp.tile([128, 1], f32, tag="gstd")
            nc.scalar.activation(gstd[:], ssd[:], Act.Sqrt,
                                 scale=float(D) / (ROWS * D - 1))
            gstde = wp.tile([128, 1], f32, tag="gstde")
            nc.vector.tensor_scalar(gstde[:], gstd[:], EPS, None, Alu.add)
            ginv = wp.tile([128, 1], f32, tag="ginv")
            nc.vector.reciprocal(ginv[:], gstde[:])
            dwg = wp.tile([128, 1], f32, tag="dwg")
            nc.vector.tensor_tensor(dwg[:], ginv[:], dw_sb[:], Alu.mult)

            # ---- phase 4: noise_std ----
            for rt in range(RT):
                crow = wp.tile([128, 1], f32, tag="crow")
                nc.vector.scalar_tensor_tensor(crow[:], dstat[:, rt:rt + 1],
                                               gmean[:], dwg[:],
                                               Alu.subtract, Alu.mult)
                mb = wp.tile([128, 1], f32, tag="mb")
                nc.vector.tensor_tensor(mb[:], im_all[:, rt:rt + 1],
                                        ii_all[:, rt:rt + 1], Alu.mult)
                bias = wp.tile([128, 1], f32, tag="bias")
                nc.vector.tensor_tensor(bias[:], crow[:], mb[:], Alu.subtract)
                sig = wpa.tile([128, D], f32, tag="sig")
                nc.scalar.activation(sig[:], inf_sb[rt][:], Act.Sigmoid,
                                     bias=bias[:], scale=ii_all[:, rt:rt + 1])
                noi = wpa.tile([128, D], f32, tag="noi")
                nc.vector.tensor_scalar(noi[:], sig[:], MAX_STD - MIN_STD, MIN_STD,
                                        Alu.mult, Alu.add)
                nc.sync.dma_start(noi_d[rt * 128:(rt + 1) * 128, :], noi[:])

    nc.compile()
    return nc


def _prep_inputs(features, memory_bank, influence_weight, distance_weight):
    x = np.ascontiguousarray(features.reshape(ROWS, D).astype(np.float32))
    m = np.ascontiguousarray(memory_bank.astype(np.float32))
    m2_64 = np.einsum("md,md->m", m.astype(np.float64), m.astype(np.float64))
    m2h = m2_64.astype(np.float16)
    m2l = (m2_64 - m2h.astype(np.float64)).astype(np.float16)
    mta = np.empty((KD, M), np.float16)
    mta[:D] = m.T.astype(np.float16)
    mta[D] = -m2h
    mta[D + 1] = -m2l
    mta = np.ascontiguousarray(mta)
    iw_rep = np.ascontiguousarray(
        np.broadcast_to(influence_weight.astype(np.float32), (128, D)))
    dw_rep = np.full((128, 1), np.float32(np.asarray(distance_weight).reshape(-1)[0]),
                     np.float32)
    cbase = np.ascontiguousarray(
        np.broadcast_to((np.arange(NCAND, dtype=np.float32) // 8) * CH,
                        (128, NCAND))).astype(np.float32)
    in_maps = []
    for c in range(NCORES):
        xs = x[c * RPC:(c + 1) * RPC]
        xa = np.empty((KD, RPC), np.float16)
        xa[:D] = (2.0 * xs.T).astype(np.float16)
        xa[D] = 1.0
        xa[D + 1] = 1.0
        in_maps.append({
            "xa": np.ascontiguousarray(xa),
            "xnat": np.ascontiguousarray(xs),
            "mta": mta,
            "bank": m,
            "iw": iw_rep,
            "dw": dw_rep,
            "cbase": cbase,
        })
    return in_maps


def kernel(features, memory_bank, influence_weight, distance_weight,
           _want_trace=False):
    from concourse.bass_utils import run_bass_kernel_spmd

    if "nc" not in _cache:
        _cache["nc"] = _build_nc()
    nc = _cache["nc"]

    in_maps = _prep_inputs(np.asarray(features), np.asarray(memory_bank),
                           np.asarray(influence_weight),
                           np.asarray(distance_weight))
    res = run_bass_kernel_spmd(nc, in_maps, core_ids=list(range(NCORES)),
                               trace=_want_trace)
    outs = res.results
    influence = np.concatenate([o["influence"] for o in outs], 0).reshape(B, N, D)
    noise = np.concatenate([o["noise"] for o in outs], 0).reshape(B, N, D)
    dist = np.concatenate([o["dist"] for o in outs], 0).reshape(B, N, KNN)
    if _want_trace:
        _cache["last_result"] = res
    return influence, noise, dist
